# revision 11
# baseline (speedup 1.0000x reference)
"""GQA attention kernel for Trainium2 (8 NeuronCores).

Sharding: core = b*4 + g  (b = batch 0..1, g = kv-group 0..3).
Each core handles one batch element and one kv head (4 query heads),
computes q/k/v projections + RoPE + causal attention + a partial o_proj
(columns of Wo for its 4 heads). Host sums the 4 bf16 partials per batch.

Per-core pipeline (chunk c = 512 query positions, ki-tile t = 128 keys):
  proj:    wcat.T @ xT per chunk -> PSUM; RoPE on DVE (SBUF pre-copy);
           v copied to bf16 vT then DMA-XBAR-transposed to natural [ki,64].
  scores:  kT row-packed pairs (2 heads concurrent via tile_position) ->
           PSUM [128,1024] per head-pair, f32r.
  exp:     one ACT instruction per (tile, pair) over the valid strided
           region, scale=1/8, output bf16 into at [128, 2048] (4 heads).
  mask:    diagonal tiles: one DVE bf16 multiply zeroes the 4 causal
           triangles (strided AP over the 4 head regions).
  av:      per head, v_aug (64 v cols + ones col, M=65) stationary bf16;
           row 64 accumulates the softmax denominator (baseline-proven
           full-array mode; col-tiled modes hang/corrupt this PE stream).
           Two head-pair passes per chunk keep PSUM within 8 banks.
  norm:    DVE reciprocal + gpsimd partition_broadcast + DVE multiply
           -> oT [128, S] f32r per head pair.
  o_proj:  oT.T @ woT -> PSUM -> DVE copy (bf16) -> DMA out.
"""

import numpy as np

B, S, D = 2, 2048, 1024
NH, NKV, HD = 16, 4, 64
SCALE = 1.0 / 8.0
ROPE_BASE = 10000.0

SC = 512  # q-chunk size
NC_CHUNKS = S // SC  # 4

LAST_RESULT = None
LAST_IN_MAPS = None
_PROG = None


def _build_program():
    from contextlib import ExitStack

    import concourse.bass as bass  # noqa: F401
    import concourse.tile as tile
    from concourse import bacc, mybir

    f32 = mybir.dt.float32
    f32r = mybir.dt.float32r
    bf16 = mybir.dt.bfloat16
    EXP = mybir.ActivationFunctionType.Exp

    nc = bacc.Bacc(trn_type="TRN2")

    xTc_d = nc.dram_tensor("xTc", [4 * D, SC], f32r, kind="ExternalInput")
    wcat_d = nc.dram_tensor("wcat", [D, 384], f32r, kind="ExternalInput")
    woT_d = nc.dram_tensor("woT", [256, D], f32r, kind="ExternalInput")
    cos_d = nc.dram_tensor("cosT", [128, S], f32, kind="ExternalInput")
    sin_d = nc.dram_tensor("sinT", [128, S], f32, kind="ExternalInput")
    maskq_d = nc.dram_tensor("maskq", [128, 512], bf16, kind="ExternalInput")
    ones_d = nc.dram_tensor("ones", [128, 8], bf16, kind="ExternalInput")
    id_d = nc.dram_tensor("ident", [64, 64], f32, kind="ExternalInput")
    y_d = nc.dram_tensor("y", [S, D], bf16, kind="ExternalOutput")

    with tile.TileContext(nc) as tc, ExitStack() as ctx:
        const = ctx.enter_context(tc.tile_pool(name="const", bufs=1))
        pers = ctx.enter_context(tc.tile_pool(name="pers", bufs=1))

        maskq_sb = const.tile([128, 512], bf16, tag="maskq")
        ones_sb = const.tile([128, 8], bf16, tag="ones")
        id_sb = const.tile([64, 64], f32, tag="ident")
        nc.sync.dma_start(maskq_sb, maskq_d[:, :])
        nc.sync.dma_start(ones_sb, ones_d[:, :])
        nc.sync.dma_start(id_sb, id_d[:, :])

        w_sb = []
        for e in range(8):
            t = const.tile([128, 384], f32r, tag=f"w{e}")
            nc.sync.dma_start(t, wcat_d[e * 128 : (e + 1) * 128, :])
            w_sb.append(t)

        cos_sb = const.tile([128, S], f32, tag="cos")
        sin_sb = const.tile([128, S], f32, tag="sin")
        xT_sb = [pers.tile([128, S], f32r, tag=f"xT{e}", name=f"xT{e}") for e in range(8)]
        nc.sync.dma_start(cos_sb, cos_d[:, :])
        nc.sync.dma_start(sin_sb, sin_d[:, :])
        # chunk-major contiguous x slices: chunk 0 lands first so compute starts early
        for c in range(NC_CHUNKS):
            cs = slice(c * SC, (c + 1) * SC)
            for e in range(8):
                nc.sync.dma_start(
                    xT_sb[e][:, cs],
                    xTc_d[c * D + e * 128 : c * D + (e + 1) * 128, :],
                )
        woT_sb = []
        for p in range(2):
            t = const.tile([128, D], f32r, tag=f"wo{p}")
            nc.sync.dma_start(t, woT_d[p * 128 : (p + 1) * 128, :])
            woT_sb.append(t)

        qT_sb = [pers.tile([128, S], f32r, tag=f"qT{m}", name=f"qT{m}") for m in range(2)]
        kT_sb = pers.tile([128, S], f32r, tag="kT")
        vTb_sb = pers.tile([64, S], f32, tag="vTb")
        vnat = [pers.tile([128, 65], bf16, tag=f"vn{t}", name=f"vn{t}") for t in range(S // 128)]
        oT_sb = [pers.tile([128, S], f32r, tag=f"oT{p}", name=f"oT{p}") for p in range(2)]
        for t in range(S // 128):
            nc.vector.tensor_copy(vnat[t][:, 64:65], ones_sb[:, 0:1])

        psc = ctx.enter_context(tc.tile_pool(name="psc", bufs=2, space="PSUM"))
        pav = ctx.enter_context(tc.tile_pool(name="pav", bufs=1, space="PSUM"))
        pmx = ctx.enter_context(tc.tile_pool(name="pmx", bufs=2, space="PSUM"))
        rsc = ctx.enter_context(tc.tile_pool(name="rsc", bufs=2))
        atp = ctx.enter_context(tc.tile_pool(name="atp", bufs=3))
        nrm = ctx.enter_context(tc.tile_pool(name="nrm", bufs=2))
        yp = ctx.enter_context(tc.tile_pool(name="yp", bufs=4))

        def rope(ps_ap, nparts, cs, out_ap, cname):
            # out = ps*cos + rot_half(ps)*sin_signed on [nparts, SC]
            qr = rsc.tile([128, SC], f32, tag="qraw", name=f"qr_{cname}")
            tmp = rsc.tile([128, SC], f32, tag="tmp", name=f"tmp_{cname}")
            nc.vector.tensor_copy(qr[0:nparts, :], ps_ap)  # PSUM -> SBUF once
            for b0 in range(0, nparts, 64):
                nc.vector.tensor_copy(tmp[b0 : b0 + 32, :], qr[b0 + 32 : b0 + 64, :])
                nc.vector.tensor_copy(tmp[b0 + 32 : b0 + 64, :], qr[b0 : b0 + 32, :])
            nc.vector.tensor_mul(
                tmp[0:nparts, :], tmp[0:nparts, :], sin_sb[0:nparts, cs]
            )
            nc.vector.tensor_mul(qr[0:nparts, :], qr[0:nparts, :], cos_sb[0:nparts, cs])
            nc.vector.tensor_add(out_ap, qr[0:nparts, :], tmp[0:nparts, :])

        def proj_gen(c):
            cs = slice(c * SC, (c + 1) * SC)
            for m in (2, 0, 1):
                ps = pmx.tile([128, SC], f32, tag="mx", name=f"proj{m}_{c}")
                for e in range(8):
                    nc.tensor.matmul(
                        ps,
                        w_sb[e][:, m * 128 : (m + 1) * 128],
                        xT_sb[e][:, cs],
                        start=(e == 0),
                        stop=(e == 7),
                    )
                if m == 2:
                    rope(ps[0:64, :], 64, cs, kT_sb[0:64, cs], f"k{c}")
                    nc.vector.tensor_copy(kT_sb[64:128, cs], kT_sb[0:64, cs])
                    nc.vector.tensor_copy(vTb_sb[:, cs], ps[64:128, :])
                    yield
                    for j in range(4):
                        t = 4 * c + j
                        pst = pmx.tile([128, SC], f32, tag="mx", name=f"tr{t}")
                        nc.tensor.transpose(
                            pst[:, 0:64],
                            vTb_sb[:, t * 128 : (t + 1) * 128],
                            id_sb,
                        )
                        nc.vector.tensor_copy(vnat[t][:, 0:64], pst[:, 0:64])
                        if j == 1:
                            yield
                    yield
                else:
                    rope(ps[:, :], 128, cs, qT_sb[m][:, cs], f"q{m}_{c}")
                    yield

        def oproj_gen(c):
            for st4 in range(4):
                st = 4 * c + st4
                for e2 in range(2):
                    ps = pmx.tile([128, SC], f32, tag="mx", name=f"op{st}_{e2}")
                    for p in range(2):
                        nc.tensor.matmul(
                            ps,
                            oT_sb[p][:, st * 128 : (st + 1) * 128],
                            woT_sb[p][:, e2 * SC : (e2 + 1) * SC],
                            start=(p == 0),
                            stop=(p == 1),
                        )
                    yt = yp.tile([128, SC], bf16, tag="y", name=f"y{st}_{e2}")
                    if (st + e2) % 2 == 0:
                        nc.vector.tensor_copy(yt, ps)
                    else:
                        nc.scalar.copy(yt, ps)
                    nc.sync.dma_start(
                        y_d[st * 128 : (st + 1) * 128, e2 * SC : (e2 + 1) * SC],
                        yt,
                    )
                    yield

        def roundrobin(gens):
            gens = list(gens)
            while gens:
                g = gens.pop(0)
                try:
                    next(g)
                    gens.append(g)
                except StopIteration:
                    pass

        def take(gens):
            # emit one filler group from the rotating generator list
            while gens:
                g = gens[0]
                try:
                    next(g)
                    gens.append(gens.pop(0))
                    return True
                except StopIteration:
                    gens.pop(0)
            return False

        for c in range(NC_CHUNKS):
            cs = slice(c * SC, (c + 1) * SC)
            if c == 0:
                roundrobin([proj_gen(0)])
            fillers = []
            if c >= 1:
                fillers.append(oproj_gen(c - 1))
            if c < NC_CHUNKS - 1:
                fillers.append(proj_gen(c + 1))

            # ---------------- attention for chunk c -------------------------
            nt = 4 * c + 4
            n_tiles = 2 * nt
            n_groups = (8 if c >= 1 else 0) + (8 if c < NC_CHUNKS - 1 else 0)
            emit_every = max(1, n_tiles // (n_groups + 1))
            tile_idx = 0
            for pss in range(2):
                ots = [
                    pav.tile([128, SC], f32, tag=f"ot{hh}", name=f"ot{hh}_{pss}_{c}")
                    for hh in range(2)
                ]
                for t in range(nt):
                    j = t - 4 * c  # >= 0 means diagonal-band tile
                    off = 128 * j if j > 0 else 0
                    att = atp.tile(
                        [128, 2 * SC], bf16, tag="at", name=f"at_{c}_{pss}_{t}"
                    )
                    ps = psc.tile(
                        [128, 2 * SC], f32, tag="sc", name=f"sc_{c}_{pss}_{t}"
                    )
                    moff = min(off, SC - 256)  # f32r needs N>=256 for 1 cyc/row
                    for hh in range(2):
                        nc.tensor.matmul(
                            ps[:, hh * SC + moff : (hh + 1) * SC],
                            kT_sb[hh * 64 : (hh + 1) * 64, t * 128 : (t + 1) * 128],
                            qT_sb[pss][
                                hh * 64 : (hh + 1) * 64, c * SC + moff : (c + 1) * SC
                            ],
                            start=True,
                            stop=True,
                        )
                    ps3 = ps[:, :].rearrange("p (h q) -> p h q", h=2)[:, :, off:SC]
                    at3 = att[:, :].rearrange("p (h q) -> p h q", h=2)[:, :, off:SC]
                    nc.scalar.activation(at3, ps3, EXP, scale=SCALE)
                    if j >= 0:
                        atm = att[:, :].rearrange("p (h q) -> p h q", h=2)[
                            :, :, off : off + 128
                        ]
                        mq = maskq_sb[:, 0:256].rearrange("p (h q) -> p h q", h=2)
                        nc.vector.tensor_mul(atm, atm, mq)
                    for hh in range(2):
                        nc.tensor.matmul(
                            ots[hh][0:65, off:SC],
                            vnat[t][:, 0:65],
                            att[:, hh * SC + off : (hh + 1) * SC],
                            start=(t == 0),
                            stop=(t == nt - 1),
                            skip_group_check=True,
                        )
                    tile_idx += 1
                    if tile_idx % emit_every == 0:
                        take(fillers)

                # normalize: oT[d, qi] = ot[d, qi] / ot[64, qi]
                for hh in range(2):
                    zr = nrm.tile([1, SC], f32, tag=f"zr{hh}", name=f"zr{hh}_{pss}_{c}")
                    nc.vector.reciprocal(zr, ots[hh][64:65, :])
                    rb = nrm.tile(
                        [64, SC], f32, tag=f"rb{hh}", name=f"rb{hh}_{pss}_{c}"
                    )
                    nc.gpsimd.partition_broadcast(rb, zr)
                    nc.vector.tensor_mul(
                        oT_sb[pss][hh * 64 : (hh + 1) * 64, cs],
                        ots[hh][0:64, :],
                        rb,
                    )
            while take(fillers):
                pass
        roundrobin([oproj_gen(NC_CHUNKS - 1)])

    nc.compile()
    return nc


def _host_constants():
    import ml_dtypes

    inv = 1.0 / (ROPE_BASE ** (np.arange(0, HD, 2, dtype=np.float64) / HD))
    freqs = np.outer(np.arange(S, dtype=np.float64), inv)  # [S, 32]
    emb = np.concatenate([freqs, freqs], axis=-1)  # [S, 64]
    cos = np.cos(emb).astype(np.float32).T  # [64, S]
    sin = np.sin(emb).astype(np.float32).T
    sgn = np.concatenate([-np.ones((32, 1)), np.ones((32, 1))]).astype(np.float32)
    sin_signed = sin * sgn
    cos128 = np.ascontiguousarray(np.concatenate([cos, cos], axis=0))
    sin128 = np.ascontiguousarray(np.concatenate([sin_signed, sin_signed], axis=0))
    ki = np.arange(128)[:, None]
    qi = np.arange(128)[None, :]
    tri = (ki <= qi).astype(np.float32)  # 1 where valid (ki <= qi)
    maskq = np.tile(tri, (1, 4)).astype(ml_dtypes.bfloat16)  # [128, 512]
    ones = np.ones((128, 8), dtype=ml_dtypes.bfloat16)
    return cos128, sin128, maskq, ones


def kernel(x, Wq, Wk, Wv, Wo):
    global LAST_RESULT, _PROG, LAST_IN_MAPS
    from concourse import bass_utils

    x = np.asarray(x, dtype=np.float32)
    Wq = np.asarray(Wq, dtype=np.float32)
    Wk = np.asarray(Wk, dtype=np.float32)
    Wv = np.asarray(Wv, dtype=np.float32)
    Wo = np.asarray(Wo, dtype=np.float32)

    if _PROG is None:
        _PROG = _build_program()
    nc = _PROG

    cos128, sin128, maskq, ones = _host_constants()
    import ml_dtypes

    ident = np.eye(64, dtype=np.float32)
    WoT = np.ascontiguousarray(Wo.T)  # [c, e]
    Wqh = Wq.reshape(NH, HD, D)
    Wkh = Wk.reshape(NKV, HD, D)
    Wvh = Wv.reshape(NKV, HD, D)

    in_maps = []
    for core in range(8):
        b, g = core // 4, core % 4
        xTc = np.ascontiguousarray(
            x[b].T.reshape(D, NC_CHUNKS, SC).transpose(1, 0, 2).reshape(
                NC_CHUNKS * D, SC
            )
        )
        wcat = np.concatenate(
            [Wqh[4 * g : 4 * g + 4].reshape(4 * HD, D), Wkh[g], Wvh[g]], axis=0
        )  # [384, D]
        wcatT = np.ascontiguousarray(wcat.T)  # [D, 384]
        woT_shard = np.ascontiguousarray(WoT[g * 256 : (g + 1) * 256, :])
        in_maps.append(
            {
                "xTc": xTc,
                "wcat": wcatT,
                "woT": woT_shard,
                "cosT": cos128,
                "sinT": sin128,
                "maskq": maskq,
                "ones": ones,
                "ident": ident,
            }
        )

    LAST_IN_MAPS = in_maps
    res = bass_utils.run_bass_kernel_spmd(nc, in_maps, core_ids=list(range(8)))
    LAST_RESULT = res
    ys = [np.asarray(m["y"], dtype=np.float32) for m in res.results]
    out = np.stack(
        [ys[0] + ys[1] + ys[2] + ys[3], ys[4] + ys[5] + ys[6] + ys[7]], axis=0
    )
    return out


def benchmark(n_iters=60):
    """Steady-state per-execution device time.

    Dispatches the jitted bass_exec repeatedly WITHOUT donation chaining so
    executions pipeline back-to-back on device; reports the slope
    (T(N2)-T(N1))/(N2-N1), which cancels fixed dispatch/transfer overhead.
    """
    import time

    import jax
    import numpy as np
    from jax.experimental.shard_map import shard_map
    from jax.sharding import Mesh, NamedSharding, PartitionSpec

    import concourse.mybir as mybir
    from concourse.bass2jax import (
        _bass_exec_p,
        install_neuronx_cc_hook,
        partition_id_tensor,
    )

    assert _PROG is not None and LAST_IN_MAPS is not None, "run kernel() first"
    nc = _PROG
    in_maps = LAST_IN_MAPS
    n_cores = 8

    install_neuronx_cc_hook()
    partition_name = nc.partition_id_tensor.name if nc.partition_id_tensor else None
    in_names, out_names, out_avals, zero_outs = [], [], [], []
    for alloc in nc.m.functions[0].allocations:
        if not isinstance(alloc, mybir.MemoryLocationSet):
            continue
        name = alloc.memorylocations[0].name
        if alloc.kind == "ExternalInput":
            if name != partition_name:
                in_names.append(name)
        elif alloc.kind == "ExternalOutput":
            dt = mybir.dt.np(alloc.dtype)
            out_avals.append(jax.core.ShapedArray(tuple(alloc.tensor_shape), dt))
            out_names.append(name)
            zero_outs.append(np.zeros(tuple(alloc.tensor_shape), dt))
    n_params = len(in_names)
    bind_in_names = list(in_names) + list(out_names)
    if partition_name is not None:
        bind_in_names.append(partition_name)

    def _body(*args):
        operands = list(args)
        if partition_name is not None:
            operands.append(partition_id_tensor())
        outs = _bass_exec_p.bind(
            *operands,
            out_avals=tuple(out_avals),
            in_names=tuple(bind_in_names),
            out_names=tuple(out_names),
            lowering_input_output_aliases=(),
            sim_require_finite=True,
            sim_require_nnan=True,
            nc=nc,
        )
        return tuple(outs)

    devices = jax.devices()[:n_cores]
    mesh = Mesh(np.asarray(devices), ("core",))
    n_outs = len(out_names)
    in_specs = (PartitionSpec("core"),) * (n_params + n_outs)
    out_specs = (PartitionSpec("core"),) * n_outs
    fn = jax.jit(
        shard_map(
            _body, mesh=mesh, in_specs=in_specs, out_specs=out_specs, check_rep=False
        ),
        keep_unused=True,
    )
    per_core = [[np.asarray(m[name]) for name in in_names] for m in in_maps]
    concat_in = [
        np.concatenate([per_core[c][i] for c in range(n_cores)], axis=0)
        for i in range(n_params)
    ]
    concat_zeros = [
        np.zeros((n_cores * z.shape[0], *z.shape[1:]), z.dtype) for z in zero_outs
    ]
    sh = NamedSharding(mesh, PartitionSpec("core"))
    params_dev = [jax.device_put(a, sh) for a in concat_in]
    z = [jax.device_put(a, sh) for a in concat_zeros]
    outs = None
    for _ in range(3):
        outs = fn(*params_dev, *z)
    jax.block_until_ready(outs)

    def run(n):
        keep = None
        t0 = time.perf_counter()
        for _ in range(n):
            keep = fn(*params_dev, *z)
        jax.block_until_ready(keep)
        return time.perf_counter() - t0

    n1, n2 = max(5, n_iters // 5), n_iters
    t1 = run(n1)
    t2 = run(n2)
    per_iter = (t2 - t1) / (n2 - n1)
    print(
        f"benchmark: T({n1})={t1 * 1e3:.2f}ms T({n2})={t2 * 1e3:.2f}ms "
        f"slope={per_iter * 1e6:.1f}us/iter"
    )
    return per_iter


# revision 12
# speedup vs baseline: 1.3017x; 1.3017x over previous
"""GQA attention kernel for Trainium2 (8 NeuronCores).

Sharding: core = b*4 + g  (b = batch 0..1, g = kv-group 0..3).
Each core handles one batch element and one kv head (4 query heads),
computes q/k/v projections + RoPE + causal attention + a partial o_proj
(columns of Wo for its 4 heads). Host sums the 4 bf16 partials per batch.

Per-core pipeline (chunk c = 512 query positions, ki-tile t = 128 keys):
  proj:    wcat.T @ xT per chunk -> PSUM; RoPE on DVE (SBUF pre-copy);
           v copied to f32 vT then PE-transposed to natural [ki, 64] bf16.
  scores:  kT row-packed pairs (2 heads concurrent via tile_position) ->
           PSUM [128,1024] per head-pair, f32r.
  exp:     one ACT instruction per (tile, pair) over the valid strided
           region, scale=1/8, output bf16 into at [128, 2048] (4 heads).
  mask:    diagonal tiles: one DVE bf16 multiply zeroes the 4 causal
           triangles (strided AP over the 4 head regions).
  av:      per head, v_aug (64 v cols + ones col, M=65) stationary bf16;
           row 64 accumulates the softmax denominator (baseline-proven
           full-array mode; col-tiled modes hang/corrupt this PE stream).
           Two head-pair passes per chunk keep PSUM within 8 banks.
  norm:    DVE reciprocal + gpsimd partition_broadcast + DVE multiply
           -> oT [128, S] f32r per head pair.
  o_proj:  oT.T @ woT -> PSUM -> DVE/ACT copy (bf16) -> DMA out.

  Engine queues are strict FIFO, so o_proj(c-1) and proj(c+1) instruction
  groups are interleaved into attention(c)'s ACT-bound tile loop via
  generator round-robin to keep PE busy (sim span 245 -> 163 us).
"""

import numpy as np

B, S, D = 2, 2048, 1024
NH, NKV, HD = 16, 4, 64
SCALE = 1.0 / 8.0
ROPE_BASE = 10000.0

SC = 512  # q-chunk size
NC_CHUNKS = S // SC  # 4

LAST_RESULT = None
LAST_IN_MAPS = None
_PROG = None


def _build_program():
    from contextlib import ExitStack

    import concourse.bass as bass  # noqa: F401
    import concourse.tile as tile
    from concourse import bacc, mybir

    f32 = mybir.dt.float32
    f32r = mybir.dt.float32r
    bf16 = mybir.dt.bfloat16
    EXP = mybir.ActivationFunctionType.Exp

    nc = bacc.Bacc(trn_type="TRN2")

    xTc_d = nc.dram_tensor("xTc", [4 * D, SC], f32r, kind="ExternalInput")
    wcat_d = nc.dram_tensor("wcat", [D, 384], f32r, kind="ExternalInput")
    woT_d = nc.dram_tensor("woT", [256, D], f32r, kind="ExternalInput")
    cos_d = nc.dram_tensor("cosT", [128, S], f32, kind="ExternalInput")
    sin_d = nc.dram_tensor("sinT", [128, S], f32, kind="ExternalInput")
    maskq_d = nc.dram_tensor("maskq", [128, 512], bf16, kind="ExternalInput")
    ones_d = nc.dram_tensor("ones", [128, 8], bf16, kind="ExternalInput")
    id_d = nc.dram_tensor("ident", [64, 64], f32, kind="ExternalInput")
    y_d = nc.dram_tensor("y", [S, D], bf16, kind="ExternalOutput")

    with tile.TileContext(nc) as tc, ExitStack() as ctx:
        const = ctx.enter_context(tc.tile_pool(name="const", bufs=1))
        pers = ctx.enter_context(tc.tile_pool(name="pers", bufs=1))

        maskq_sb = const.tile([128, 512], bf16, tag="maskq")
        ones_sb = const.tile([128, 8], bf16, tag="ones")
        id_sb = const.tile([64, 64], f32, tag="ident")
        nc.sync.dma_start(maskq_sb, maskq_d[:, :])
        nc.sync.dma_start(ones_sb, ones_d[:, :])
        nc.sync.dma_start(id_sb, id_d[:, :])

        w_sb = []
        for e in range(8):
            t = const.tile([128, 384], f32r, tag=f"w{e}")
            nc.sync.dma_start(t, wcat_d[e * 128 : (e + 1) * 128, :])
            w_sb.append(t)

        cos_sb = const.tile([128, S], f32, tag="cos")
        sin_sb = const.tile([128, S], f32, tag="sin")
        xT_sb = [pers.tile([128, S], f32r, tag=f"xT{e}", name=f"xT{e}") for e in range(8)]
        nc.sync.dma_start(cos_sb, cos_d[:, :])
        nc.sync.dma_start(sin_sb, sin_d[:, :])
        # chunk-major contiguous x slices: chunk 0 lands first so compute starts early
        for c in range(NC_CHUNKS):
            cs = slice(c * SC, (c + 1) * SC)
            for e in range(8):
                nc.sync.dma_start(
                    xT_sb[e][:, cs],
                    xTc_d[c * D + e * 128 : c * D + (e + 1) * 128, :],
                )
        woT_sb = []
        for p in range(2):
            t = const.tile([128, D], f32r, tag=f"wo{p}")
            nc.sync.dma_start(t, woT_d[p * 128 : (p + 1) * 128, :])
            woT_sb.append(t)

        qT_sb = [pers.tile([128, S], f32r, tag=f"qT{m}", name=f"qT{m}") for m in range(2)]
        kT_sb = pers.tile([128, S], f32r, tag="kT")
        vTb_sb = pers.tile([64, S], f32, tag="vTb")
        vnat = [pers.tile([128, 65], bf16, tag=f"vn{t}", name=f"vn{t}") for t in range(S // 128)]
        oT_sb = [pers.tile([128, S], f32r, tag=f"oT{p}", name=f"oT{p}") for p in range(2)]
        for t in range(S // 128):
            nc.vector.tensor_copy(vnat[t][:, 64:65], ones_sb[:, 0:1])

        psc = ctx.enter_context(tc.tile_pool(name="psc", bufs=2, space="PSUM"))
        pav = ctx.enter_context(tc.tile_pool(name="pav", bufs=1, space="PSUM"))
        pmx = ctx.enter_context(tc.tile_pool(name="pmx", bufs=2, space="PSUM"))
        rsc = ctx.enter_context(tc.tile_pool(name="rsc", bufs=2))
        atp = ctx.enter_context(tc.tile_pool(name="atp", bufs=3))
        nrm = ctx.enter_context(tc.tile_pool(name="nrm", bufs=2))
        yp = ctx.enter_context(tc.tile_pool(name="yp", bufs=4))

        def rope(ps_ap, nparts, cs, out_ap, cname):
            # out = ps*cos + rot_half(ps)*sin_signed on [nparts, SC]
            qr = rsc.tile([128, SC], f32, tag="qraw", name=f"qr_{cname}")
            tmp = rsc.tile([128, SC], f32, tag="tmp", name=f"tmp_{cname}")
            nc.vector.tensor_copy(qr[0:nparts, :], ps_ap)  # PSUM -> SBUF once
            for b0 in range(0, nparts, 64):
                nc.vector.tensor_copy(tmp[b0 : b0 + 32, :], qr[b0 + 32 : b0 + 64, :])
                nc.vector.tensor_copy(tmp[b0 + 32 : b0 + 64, :], qr[b0 : b0 + 32, :])
            nc.vector.tensor_mul(
                tmp[0:nparts, :], tmp[0:nparts, :], sin_sb[0:nparts, cs]
            )
            nc.vector.tensor_mul(qr[0:nparts, :], qr[0:nparts, :], cos_sb[0:nparts, cs])
            nc.vector.tensor_add(out_ap, qr[0:nparts, :], tmp[0:nparts, :])

        def proj_gen(c):
            cs = slice(c * SC, (c + 1) * SC)
            for m in (2, 0, 1):
                ps = pmx.tile([128, SC], f32, tag="mx", name=f"proj{m}_{c}")
                for e in range(8):
                    nc.tensor.matmul(
                        ps,
                        w_sb[e][:, m * 128 : (m + 1) * 128],
                        xT_sb[e][:, cs],
                        start=(e == 0),
                        stop=(e == 7),
                    )
                if m == 2:
                    rope(ps[0:64, :], 64, cs, kT_sb[0:64, cs], f"k{c}")
                    nc.vector.tensor_copy(kT_sb[64:128, cs], kT_sb[0:64, cs])
                    nc.vector.tensor_copy(vTb_sb[:, cs], ps[64:128, :])
                    yield
                    for j in range(4):
                        t = 4 * c + j
                        pst = pmx.tile([128, SC], f32, tag="mx", name=f"tr{t}")
                        nc.tensor.transpose(
                            pst[:, 0:64],
                            vTb_sb[:, t * 128 : (t + 1) * 128],
                            id_sb,
                        )
                        nc.vector.tensor_copy(vnat[t][:, 0:64], pst[:, 0:64])
                        if j == 1:
                            yield
                    yield
                else:
                    rope(ps[:, :], 128, cs, qT_sb[m][:, cs], f"q{m}_{c}")
                    yield

        def oproj_gen(c):
            for st4 in range(4):
                st = 4 * c + st4
                for e2 in range(2):
                    ps = pmx.tile([128, SC], f32, tag="mx", name=f"op{st}_{e2}")
                    for p in range(2):
                        nc.tensor.matmul(
                            ps,
                            oT_sb[p][:, st * 128 : (st + 1) * 128],
                            woT_sb[p][:, e2 * SC : (e2 + 1) * SC],
                            start=(p == 0),
                            stop=(p == 1),
                        )
                    yt = yp.tile([128, SC], bf16, tag="y", name=f"y{st}_{e2}")
                    if (st + e2) % 2 == 0:
                        nc.vector.tensor_copy(yt, ps)
                    else:
                        nc.scalar.copy(yt, ps)
                    nc.sync.dma_start(
                        y_d[st * 128 : (st + 1) * 128, e2 * SC : (e2 + 1) * SC],
                        yt,
                    )
                    yield

        def roundrobin(gens):
            gens = list(gens)
            while gens:
                g = gens.pop(0)
                try:
                    next(g)
                    gens.append(g)
                except StopIteration:
                    pass

        def take(gens):
            # emit one filler group from the rotating generator list
            while gens:
                g = gens[0]
                try:
                    next(g)
                    gens.append(gens.pop(0))
                    return True
                except StopIteration:
                    gens.pop(0)
            return False

        for c in range(NC_CHUNKS):
            cs = slice(c * SC, (c + 1) * SC)
            if c == 0:
                roundrobin([proj_gen(0)])
            fillers = []
            if c >= 1:
                fillers.append(oproj_gen(c - 1))
            if c < NC_CHUNKS - 1:
                fillers.append(proj_gen(c + 1))

            # ---------------- attention for chunk c -------------------------
            nt = 4 * c + 4
            n_tiles = 2 * nt
            n_groups = (8 if c >= 1 else 0) + (8 if c < NC_CHUNKS - 1 else 0)
            emit_every = max(1, n_tiles // (n_groups + 1))
            tile_idx = 0
            for pss in range(2):
                ots = [
                    pav.tile([128, SC], f32, tag=f"ot{hh}", name=f"ot{hh}_{pss}_{c}")
                    for hh in range(2)
                ]
                for t in range(nt):
                    j = t - 4 * c  # >= 0 means diagonal-band tile
                    off = 128 * j if j > 0 else 0
                    att = atp.tile(
                        [128, 2 * SC], bf16, tag="at", name=f"at_{c}_{pss}_{t}"
                    )
                    ps = psc.tile(
                        [128, 2 * SC], f32, tag="sc", name=f"sc_{c}_{pss}_{t}"
                    )
                    moff = min(off, SC - 256)  # f32r needs N>=256 for 1 cyc/row
                    for hh in range(2):
                        nc.tensor.matmul(
                            ps[:, hh * SC + moff : (hh + 1) * SC],
                            kT_sb[hh * 64 : (hh + 1) * 64, t * 128 : (t + 1) * 128],
                            qT_sb[pss][
                                hh * 64 : (hh + 1) * 64, c * SC + moff : (c + 1) * SC
                            ],
                            start=True,
                            stop=True,
                        )
                    ps3 = ps[:, :].rearrange("p (h q) -> p h q", h=2)[:, :, off:SC]
                    at3 = att[:, :].rearrange("p (h q) -> p h q", h=2)[:, :, off:SC]
                    nc.scalar.activation(at3, ps3, EXP, scale=SCALE)
                    if j >= 0:
                        atm = att[:, :].rearrange("p (h q) -> p h q", h=2)[
                            :, :, off : off + 128
                        ]
                        mq = maskq_sb[:, 0:256].rearrange("p (h q) -> p h q", h=2)
                        nc.vector.tensor_mul(atm, atm, mq)
                    for hh in range(2):
                        nc.tensor.matmul(
                            ots[hh][0:65, off:SC],
                            vnat[t][:, 0:65],
                            att[:, hh * SC + off : (hh + 1) * SC],
                            start=(t == 0),
                            stop=(t == nt - 1),
                            skip_group_check=True,
                        )
                    tile_idx += 1
                    if tile_idx % emit_every == 0:
                        take(fillers)

                # normalize: oT[d, qi] = ot[d, qi] / ot[64, qi]
                for hh in range(2):
                    zr = nrm.tile([1, SC], f32, tag=f"zr{hh}", name=f"zr{hh}_{pss}_{c}")
                    nc.vector.reciprocal(zr, ots[hh][64:65, :])
                    rb = nrm.tile(
                        [64, SC], f32, tag=f"rb{hh}", name=f"rb{hh}_{pss}_{c}"
                    )
                    nc.gpsimd.partition_broadcast(rb, zr)
                    nc.vector.tensor_mul(
                        oT_sb[pss][hh * 64 : (hh + 1) * 64, cs],
                        ots[hh][0:64, :],
                        rb,
                    )
            while take(fillers):
                pass
        roundrobin([oproj_gen(NC_CHUNKS - 1)])

    nc.compile()
    return nc


def _host_constants():
    import ml_dtypes

    inv = 1.0 / (ROPE_BASE ** (np.arange(0, HD, 2, dtype=np.float64) / HD))
    freqs = np.outer(np.arange(S, dtype=np.float64), inv)  # [S, 32]
    emb = np.concatenate([freqs, freqs], axis=-1)  # [S, 64]
    cos = np.cos(emb).astype(np.float32).T  # [64, S]
    sin = np.sin(emb).astype(np.float32).T
    sgn = np.concatenate([-np.ones((32, 1)), np.ones((32, 1))]).astype(np.float32)
    sin_signed = sin * sgn
    cos128 = np.ascontiguousarray(np.concatenate([cos, cos], axis=0))
    sin128 = np.ascontiguousarray(np.concatenate([sin_signed, sin_signed], axis=0))
    ki = np.arange(128)[:, None]
    qi = np.arange(128)[None, :]
    tri = (ki <= qi).astype(np.float32)  # 1 where valid (ki <= qi)
    maskq = np.tile(tri, (1, 4)).astype(ml_dtypes.bfloat16)  # [128, 512]
    ones = np.ones((128, 8), dtype=ml_dtypes.bfloat16)
    return cos128, sin128, maskq, ones


def kernel(x, Wq, Wk, Wv, Wo):
    global LAST_RESULT, _PROG, LAST_IN_MAPS
    from concourse import bass_utils

    x = np.asarray(x, dtype=np.float32)
    Wq = np.asarray(Wq, dtype=np.float32)
    Wk = np.asarray(Wk, dtype=np.float32)
    Wv = np.asarray(Wv, dtype=np.float32)
    Wo = np.asarray(Wo, dtype=np.float32)

    if _PROG is None:
        _PROG = _build_program()
    nc = _PROG

    cos128, sin128, maskq, ones = _host_constants()
    import ml_dtypes

    ident = np.eye(64, dtype=np.float32)
    WoT = np.ascontiguousarray(Wo.T)  # [c, e]
    Wqh = Wq.reshape(NH, HD, D)
    Wkh = Wk.reshape(NKV, HD, D)
    Wvh = Wv.reshape(NKV, HD, D)

    in_maps = []
    for core in range(8):
        b, g = core // 4, core % 4
        xTc = np.ascontiguousarray(
            x[b].T.reshape(D, NC_CHUNKS, SC).transpose(1, 0, 2).reshape(
                NC_CHUNKS * D, SC
            )
        )
        wcat = np.concatenate(
            [Wqh[4 * g : 4 * g + 4].reshape(4 * HD, D), Wkh[g], Wvh[g]], axis=0
        )  # [384, D]
        wcatT = np.ascontiguousarray(wcat.T)  # [D, 384]
        woT_shard = np.ascontiguousarray(WoT[g * 256 : (g + 1) * 256, :])
        in_maps.append(
            {
                "xTc": xTc,
                "wcat": wcatT,
                "woT": woT_shard,
                "cosT": cos128,
                "sinT": sin128,
                "maskq": maskq,
                "ones": ones,
                "ident": ident,
            }
        )

    LAST_IN_MAPS = in_maps
    res = bass_utils.run_bass_kernel_spmd(nc, in_maps, core_ids=list(range(8)))
    LAST_RESULT = res
    ys = [np.asarray(m["y"], dtype=np.float32) for m in res.results]
    out = np.stack(
        [ys[0] + ys[1] + ys[2] + ys[3], ys[4] + ys[5] + ys[6] + ys[7]], axis=0
    )
    return out


def benchmark(n_iters=60):
    """Steady-state per-execution device time.

    Dispatches the jitted bass_exec repeatedly WITHOUT donation chaining so
    executions pipeline back-to-back on device; reports the slope
    (T(N2)-T(N1))/(N2-N1), which cancels fixed dispatch/transfer overhead.
    """
    import time

    import jax
    import numpy as np
    from jax.experimental.shard_map import shard_map
    from jax.sharding import Mesh, NamedSharding, PartitionSpec

    import concourse.mybir as mybir
    from concourse.bass2jax import (
        _bass_exec_p,
        install_neuronx_cc_hook,
        partition_id_tensor,
    )

    assert _PROG is not None and LAST_IN_MAPS is not None, "run kernel() first"
    nc = _PROG
    in_maps = LAST_IN_MAPS
    n_cores = 8

    install_neuronx_cc_hook()
    partition_name = nc.partition_id_tensor.name if nc.partition_id_tensor else None
    in_names, out_names, out_avals, zero_outs = [], [], [], []
    for alloc in nc.m.functions[0].allocations:
        if not isinstance(alloc, mybir.MemoryLocationSet):
            continue
        name = alloc.memorylocations[0].name
        if alloc.kind == "ExternalInput":
            if name != partition_name:
                in_names.append(name)
        elif alloc.kind == "ExternalOutput":
            dt = mybir.dt.np(alloc.dtype)
            out_avals.append(jax.core.ShapedArray(tuple(alloc.tensor_shape), dt))
            out_names.append(name)
            zero_outs.append(np.zeros(tuple(alloc.tensor_shape), dt))
    n_params = len(in_names)
    bind_in_names = list(in_names) + list(out_names)
    if partition_name is not None:
        bind_in_names.append(partition_name)

    def _body(*args):
        operands = list(args)
        if partition_name is not None:
            operands.append(partition_id_tensor())
        outs = _bass_exec_p.bind(
            *operands,
            out_avals=tuple(out_avals),
            in_names=tuple(bind_in_names),
            out_names=tuple(out_names),
            lowering_input_output_aliases=(),
            sim_require_finite=True,
            sim_require_nnan=True,
            nc=nc,
        )
        return tuple(outs)

    devices = jax.devices()[:n_cores]
    mesh = Mesh(np.asarray(devices), ("core",))
    n_outs = len(out_names)
    in_specs = (PartitionSpec("core"),) * (n_params + n_outs)
    out_specs = (PartitionSpec("core"),) * n_outs
    fn = jax.jit(
        shard_map(
            _body, mesh=mesh, in_specs=in_specs, out_specs=out_specs, check_rep=False
        ),
        keep_unused=True,
    )
    per_core = [[np.asarray(m[name]) for name in in_names] for m in in_maps]
    concat_in = [
        np.concatenate([per_core[c][i] for c in range(n_cores)], axis=0)
        for i in range(n_params)
    ]
    concat_zeros = [
        np.zeros((n_cores * z.shape[0], *z.shape[1:]), z.dtype) for z in zero_outs
    ]
    sh = NamedSharding(mesh, PartitionSpec("core"))
    params_dev = [jax.device_put(a, sh) for a in concat_in]
    z = [jax.device_put(a, sh) for a in concat_zeros]
    outs = None
    for _ in range(3):
        outs = fn(*params_dev, *z)
    jax.block_until_ready(outs)

    def run(n):
        keep = None
        t0 = time.perf_counter()
        for _ in range(n):
            keep = fn(*params_dev, *z)
        jax.block_until_ready(keep)
        return time.perf_counter() - t0

    n1, n2 = max(5, n_iters // 5), n_iters
    t1 = run(n1)
    t2 = run(n2)
    per_iter = (t2 - t1) / (n2 - n1)
    print(
        f"benchmark: T({n1})={t1 * 1e3:.2f}ms T({n2})={t2 * 1e3:.2f}ms "
        f"slope={per_iter * 1e6:.1f}us/iter"
    )
    return per_iter


# revision 13
# speedup vs baseline: 4.9075x; 3.7701x over previous
"""GQA attention kernel for Trainium2 (8 NeuronCores).

Sharding: core = b*4 + g  (b = batch 0..1, g = kv-group 0..3).
Each core handles one batch element and one kv head (4 query heads),
computes q/k/v projections + RoPE + causal attention + a partial o_proj
(columns of Wo for its 4 heads). Host sums the 4 bf16 partials per batch.

Per-core pipeline (chunk c = 512 query positions, ki-tile t = 128 keys):
  proj:    wcat.T @ xT per chunk -> PSUM; RoPE on DVE (SBUF pre-copy);
           v copied to f32 vT then PE-transposed to natural [ki, 64] bf16.
  scores:  kT row-packed pairs (2 heads concurrent via tile_position) ->
           PSUM [128,1024] per head-pair, f32r.
  exp:     one ACT instruction per (tile, pair) over the valid strided
           region, scale=1/8, output bf16 into at [128, 2048] (4 heads).
  mask:    diagonal tiles: one DVE bf16 multiply zeroes the 4 causal
           triangles (strided AP over the 4 head regions).
  av:      per head, v_aug (64 v cols + ones col, M=65) stationary bf16;
           row 64 accumulates the softmax denominator (baseline-proven
           full-array mode; col-tiled modes hang/corrupt this PE stream).
           Two head-pair passes per chunk keep PSUM within 8 banks.
  norm:    DVE reciprocal + gpsimd partition_broadcast + DVE multiply
           -> oT [128, S] f32r per head pair.
  o_proj:  oT.T @ woT -> PSUM -> DVE/ACT copy (bf16) -> DMA out.

  Engine queues are strict FIFO, so o_proj(c-1) and proj(c+1) instruction
  groups are interleaved into attention(c)'s ACT-bound tile loop via
  generator round-robin to keep PE busy (sim span 245 -> 163 us).
"""

import numpy as np

B, S, D = 2, 2048, 1024
NH, NKV, HD = 16, 4, 64
SCALE = 1.0 / 8.0
ROPE_BASE = 10000.0

SC = 512  # q-chunk size
NC_CHUNKS = S // SC  # 4

LAST_RESULT = None
LAST_IN_MAPS = None
_PROG = None


def _build_program():
    from contextlib import ExitStack

    import concourse.bass as bass  # noqa: F401
    import concourse.tile as tile
    from concourse import bacc, mybir

    f32 = mybir.dt.float32
    f32r = mybir.dt.float32r
    bf16 = mybir.dt.bfloat16
    EXP = mybir.ActivationFunctionType.Exp

    nc = bacc.Bacc(trn_type="TRN2")

    xTc_d = nc.dram_tensor("xTc", [4 * D, SC], f32r, kind="ExternalInput")
    wcat_d = nc.dram_tensor("wcat", [D, 384], f32r, kind="ExternalInput")
    woT_d = nc.dram_tensor("woT", [256, D], f32r, kind="ExternalInput")
    cos_d = nc.dram_tensor("cosT", [128, S], f32, kind="ExternalInput")
    sin_d = nc.dram_tensor("sinT", [128, S], f32, kind="ExternalInput")
    maskq_d = nc.dram_tensor("maskq", [128, 512], bf16, kind="ExternalInput")
    ones_d = nc.dram_tensor("ones", [128, 8], bf16, kind="ExternalInput")
    id_d = nc.dram_tensor("ident", [64, 64], f32, kind="ExternalInput")
    y_d = nc.dram_tensor("y", [S, D], bf16, kind="ExternalOutput")

    with tile.TileContext(nc) as tc, ExitStack() as ctx:
        const = ctx.enter_context(tc.tile_pool(name="const", bufs=1))
        pers = ctx.enter_context(tc.tile_pool(name="pers", bufs=1))

        # DMA issue order = arrival order: the first proj matmuls need only
        # wcat + x chunk 0, so those go first; cos/sin (rope) next; then the
        # small attention constants, remaining x chunks, and woT (o_proj).
        w_sb = []
        for e in range(8):
            t = const.tile([128, 384], f32r, tag=f"w{e}")
            nc.sync.dma_start(t, wcat_d[e * 128 : (e + 1) * 128, :])
            w_sb.append(t)

        cos_sb = const.tile([128, S], f32, tag="cos")
        sin_sb = const.tile([128, S], f32, tag="sin")
        xT_sb = [pers.tile([128, S], f32r, tag=f"xT{e}", name=f"xT{e}") for e in range(8)]
        for e in range(8):
            nc.sync.dma_start(xT_sb[e][:, 0:SC], xTc_d[e * 128 : (e + 1) * 128, :])
        nc.sync.dma_start(cos_sb, cos_d[:, :])
        nc.sync.dma_start(sin_sb, sin_d[:, :])

        maskq_sb = const.tile([128, 512], bf16, tag="maskq")
        ones_sb = const.tile([128, 8], bf16, tag="ones")
        id_sb = const.tile([64, 64], f32, tag="ident")
        nc.sync.dma_start(maskq_sb, maskq_d[:, :])
        nc.sync.dma_start(ones_sb, ones_d[:, :])
        nc.sync.dma_start(id_sb, id_d[:, :])

        for c in range(1, NC_CHUNKS):
            cs = slice(c * SC, (c + 1) * SC)
            for e in range(8):
                nc.sync.dma_start(
                    xT_sb[e][:, cs],
                    xTc_d[c * D + e * 128 : c * D + (e + 1) * 128, :],
                )
        woT_sb = []
        for p in range(2):
            t = const.tile([128, D], f32r, tag=f"wo{p}")
            nc.sync.dma_start(t, woT_d[p * 128 : (p + 1) * 128, :])
            woT_sb.append(t)

        qT_sb = [pers.tile([128, S], f32r, tag=f"qT{m}", name=f"qT{m}") for m in range(2)]
        kT_sb = pers.tile([128, S], f32r, tag="kT")
        vTb_sb = pers.tile([64, S], f32, tag="vTb")
        vnat = [pers.tile([128, 65], bf16, tag=f"vn{t}", name=f"vn{t}") for t in range(S // 128)]
        oT_sb = [pers.tile([128, S], f32r, tag=f"oT{p}", name=f"oT{p}") for p in range(2)]
        for t in range(S // 128):
            nc.vector.tensor_copy(vnat[t][:, 64:65], ones_sb[:, 0:1])

        psc = ctx.enter_context(tc.tile_pool(name="psc", bufs=2, space="PSUM"))
        pav = ctx.enter_context(tc.tile_pool(name="pav", bufs=1, space="PSUM"))
        pmx = ctx.enter_context(tc.tile_pool(name="pmx", bufs=2, space="PSUM"))
        rsc = ctx.enter_context(tc.tile_pool(name="rsc", bufs=2))
        atp = ctx.enter_context(tc.tile_pool(name="atp", bufs=3))
        nrm = ctx.enter_context(tc.tile_pool(name="nrm", bufs=2))
        yp = ctx.enter_context(tc.tile_pool(name="yp", bufs=4))

        def rope(ps_ap, nparts, cs, out_ap, cname):
            # out = ps*cos + rot_half(ps)*sin_signed on [nparts, SC]
            qr = rsc.tile([128, SC], f32, tag="qraw", name=f"qr_{cname}")
            tmp = rsc.tile([128, SC], f32, tag="tmp", name=f"tmp_{cname}")
            nc.vector.tensor_copy(qr[0:nparts, :], ps_ap)  # PSUM -> SBUF once
            for b0 in range(0, nparts, 64):
                nc.vector.tensor_copy(tmp[b0 : b0 + 32, :], qr[b0 + 32 : b0 + 64, :])
                nc.vector.tensor_copy(tmp[b0 + 32 : b0 + 64, :], qr[b0 : b0 + 32, :])
            nc.vector.tensor_mul(
                tmp[0:nparts, :], tmp[0:nparts, :], sin_sb[0:nparts, cs]
            )
            nc.vector.tensor_mul(qr[0:nparts, :], qr[0:nparts, :], cos_sb[0:nparts, cs])
            nc.vector.tensor_add(out_ap, qr[0:nparts, :], tmp[0:nparts, :])

        def proj_gen(c):
            cs = slice(c * SC, (c + 1) * SC)
            for m in (2, 0, 1):
                ps = pmx.tile([128, SC], f32, tag="mx", name=f"proj{m}_{c}")
                for e in range(8):
                    nc.tensor.matmul(
                        ps,
                        w_sb[e][:, m * 128 : (m + 1) * 128],
                        xT_sb[e][:, cs],
                        start=(e == 0),
                        stop=(e == 7),
                    )
                if m == 2:
                    rope(ps[0:64, :], 64, cs, kT_sb[0:64, cs], f"k{c}")
                    nc.vector.tensor_copy(kT_sb[64:128, cs], kT_sb[0:64, cs])
                    nc.vector.tensor_copy(vTb_sb[:, cs], ps[64:128, :])
                    yield
                    for j in range(4):
                        t = 4 * c + j
                        pst = pmx.tile([128, SC], f32, tag="mx", name=f"tr{t}")
                        nc.tensor.transpose(
                            pst[:, 0:64],
                            vTb_sb[:, t * 128 : (t + 1) * 128],
                            id_sb,
                        )
                        nc.vector.tensor_copy(vnat[t][:, 0:64], pst[:, 0:64])
                        if j == 1:
                            yield
                    yield
                else:
                    rope(ps[:, :], 128, cs, qT_sb[m][:, cs], f"q{m}_{c}")
                    yield

        def oproj_gen(c):
            for st4 in range(4):
                st = 4 * c + st4
                for e2 in range(2):
                    ps = pmx.tile([128, SC], f32, tag="mx", name=f"op{st}_{e2}")
                    for p in range(2):
                        nc.tensor.matmul(
                            ps,
                            oT_sb[p][:, st * 128 : (st + 1) * 128],
                            woT_sb[p][:, e2 * SC : (e2 + 1) * SC],
                            start=(p == 0),
                            stop=(p == 1),
                        )
                    yt = yp.tile([128, SC], bf16, tag="y", name=f"y{st}_{e2}")
                    if (st + e2) % 2 == 0:
                        nc.vector.tensor_copy(yt, ps)
                    else:
                        nc.scalar.copy(yt, ps)
                    nc.sync.dma_start(
                        y_d[st * 128 : (st + 1) * 128, e2 * SC : (e2 + 1) * SC],
                        yt,
                    )
                    yield

        def roundrobin(gens):
            gens = list(gens)
            while gens:
                g = gens.pop(0)
                try:
                    next(g)
                    gens.append(g)
                except StopIteration:
                    pass

        def take(gens):
            # emit one filler group from the rotating generator list
            while gens:
                g = gens[0]
                try:
                    next(g)
                    gens.append(gens.pop(0))
                    return True
                except StopIteration:
                    gens.pop(0)
            return False

        for c in range(NC_CHUNKS):
            cs = slice(c * SC, (c + 1) * SC)
            if c == 0:
                roundrobin([proj_gen(0)])
            fillers = []
            if c >= 1:
                fillers.append(oproj_gen(c - 1))
            if c < NC_CHUNKS - 1:
                fillers.append(proj_gen(c + 1))

            # ---------------- attention for chunk c -------------------------
            nt = 4 * c + 4
            n_tiles = 2 * nt
            n_groups = (8 if c >= 1 else 0) + (8 if c < NC_CHUNKS - 1 else 0)
            emit_every = max(1, n_tiles // (n_groups + 1))
            tile_idx = 0
            for pss in range(2):
                ots = [
                    pav.tile([128, SC], f32, tag=f"ot{hh}", name=f"ot{hh}_{pss}_{c}")
                    for hh in range(2)
                ]
                for t in range(nt):
                    j = t - 4 * c  # >= 0 means diagonal-band tile
                    off = 128 * j if j > 0 else 0
                    att = atp.tile(
                        [128, 2 * SC], bf16, tag="at", name=f"at_{c}_{pss}_{t}"
                    )
                    ps = psc.tile(
                        [128, 2 * SC], f32, tag="sc", name=f"sc_{c}_{pss}_{t}"
                    )
                    moff = min(off, SC - 256)  # f32r needs N>=256 for 1 cyc/row
                    for hh in range(2):
                        nc.tensor.matmul(
                            ps[:, hh * SC + moff : (hh + 1) * SC],
                            kT_sb[hh * 64 : (hh + 1) * 64, t * 128 : (t + 1) * 128],
                            qT_sb[pss][
                                hh * 64 : (hh + 1) * 64, c * SC + moff : (c + 1) * SC
                            ],
                            start=True,
                            stop=True,
                        )
                    ps3 = ps[:, :].rearrange("p (h q) -> p h q", h=2)[:, :, off:SC]
                    at3 = att[:, :].rearrange("p (h q) -> p h q", h=2)[:, :, off:SC]
                    nc.scalar.activation(at3, ps3, EXP, scale=SCALE)
                    if j >= 0:
                        atm = att[:, :].rearrange("p (h q) -> p h q", h=2)[
                            :, :, off : off + 128
                        ]
                        mq = maskq_sb[:, 0:256].rearrange("p (h q) -> p h q", h=2)
                        nc.vector.tensor_mul(atm, atm, mq)
                    for hh in range(2):
                        nc.tensor.matmul(
                            ots[hh][0:65, off:SC],
                            vnat[t][:, 0:65],
                            att[:, hh * SC + off : (hh + 1) * SC],
                            start=(t == 0),
                            stop=(t == nt - 1),
                            skip_group_check=True,
                        )
                    tile_idx += 1
                    if tile_idx % emit_every == 0:
                        take(fillers)

                # normalize: oT[d, qi] = ot[d, qi] / ot[64, qi]
                for hh in range(2):
                    zr = nrm.tile([1, SC], f32, tag=f"zr{hh}", name=f"zr{hh}_{pss}_{c}")
                    nc.vector.reciprocal(zr, ots[hh][64:65, :])
                    rb = nrm.tile(
                        [64, SC], f32, tag=f"rb{hh}", name=f"rb{hh}_{pss}_{c}"
                    )
                    nc.gpsimd.partition_broadcast(rb, zr)
                    nc.vector.tensor_mul(
                        oT_sb[pss][hh * 64 : (hh + 1) * 64, cs],
                        ots[hh][0:64, :],
                        rb,
                    )
            while take(fillers):
                pass
        roundrobin([oproj_gen(NC_CHUNKS - 1)])

    nc.compile()
    return nc


def _host_constants():
    import ml_dtypes

    inv = 1.0 / (ROPE_BASE ** (np.arange(0, HD, 2, dtype=np.float64) / HD))
    freqs = np.outer(np.arange(S, dtype=np.float64), inv)  # [S, 32]
    emb = np.concatenate([freqs, freqs], axis=-1)  # [S, 64]
    cos = np.cos(emb).astype(np.float32).T  # [64, S]
    sin = np.sin(emb).astype(np.float32).T
    sgn = np.concatenate([-np.ones((32, 1)), np.ones((32, 1))]).astype(np.float32)
    sin_signed = sin * sgn
    cos128 = np.ascontiguousarray(np.concatenate([cos, cos], axis=0))
    sin128 = np.ascontiguousarray(np.concatenate([sin_signed, sin_signed], axis=0))
    ki = np.arange(128)[:, None]
    qi = np.arange(128)[None, :]
    tri = (ki <= qi).astype(np.float32)  # 1 where valid (ki <= qi)
    maskq = np.tile(tri, (1, 4)).astype(ml_dtypes.bfloat16)  # [128, 512]
    ones = np.ones((128, 8), dtype=ml_dtypes.bfloat16)
    return cos128, sin128, maskq, ones


def kernel(x, Wq, Wk, Wv, Wo):
    global LAST_RESULT, _PROG, LAST_IN_MAPS
    from concourse import bass_utils

    x = np.asarray(x, dtype=np.float32)
    Wq = np.asarray(Wq, dtype=np.float32)
    Wk = np.asarray(Wk, dtype=np.float32)
    Wv = np.asarray(Wv, dtype=np.float32)
    Wo = np.asarray(Wo, dtype=np.float32)

    if _PROG is None:
        _PROG = _build_program()
    nc = _PROG

    cos128, sin128, maskq, ones = _host_constants()
    import ml_dtypes

    ident = np.eye(64, dtype=np.float32)
    WoT = np.ascontiguousarray(Wo.T)  # [c, e]
    Wqh = Wq.reshape(NH, HD, D)
    Wkh = Wk.reshape(NKV, HD, D)
    Wvh = Wv.reshape(NKV, HD, D)

    in_maps = []
    for core in range(8):
        b, g = core // 4, core % 4
        xTc = np.ascontiguousarray(
            x[b].T.reshape(D, NC_CHUNKS, SC).transpose(1, 0, 2).reshape(
                NC_CHUNKS * D, SC
            )
        )
        wcat = np.concatenate(
            [Wqh[4 * g : 4 * g + 4].reshape(4 * HD, D), Wkh[g], Wvh[g]], axis=0
        )  # [384, D]
        wcatT = np.ascontiguousarray(wcat.T)  # [D, 384]
        woT_shard = np.ascontiguousarray(WoT[g * 256 : (g + 1) * 256, :])
        in_maps.append(
            {
                "xTc": xTc,
                "wcat": wcatT,
                "woT": woT_shard,
                "cosT": cos128,
                "sinT": sin128,
                "maskq": maskq,
                "ones": ones,
                "ident": ident,
            }
        )

    LAST_IN_MAPS = in_maps
    res = bass_utils.run_bass_kernel_spmd(nc, in_maps, core_ids=list(range(8)))
    LAST_RESULT = res
    ys = [np.asarray(m["y"], dtype=np.float32) for m in res.results]
    out = np.stack(
        [ys[0] + ys[1] + ys[2] + ys[3], ys[4] + ys[5] + ys[6] + ys[7]], axis=0
    )
    return out


def benchmark(n_iters=60):
    """Steady-state per-execution device time.

    Dispatches the jitted bass_exec repeatedly WITHOUT donation chaining so
    executions pipeline back-to-back on device; reports the slope
    (T(N2)-T(N1))/(N2-N1), which cancels fixed dispatch/transfer overhead.
    """
    import time

    import jax
    import numpy as np
    from jax.experimental.shard_map import shard_map
    from jax.sharding import Mesh, NamedSharding, PartitionSpec

    import concourse.mybir as mybir
    from concourse.bass2jax import (
        _bass_exec_p,
        install_neuronx_cc_hook,
        partition_id_tensor,
    )

    assert _PROG is not None and LAST_IN_MAPS is not None, "run kernel() first"
    nc = _PROG
    in_maps = LAST_IN_MAPS
    n_cores = 8

    install_neuronx_cc_hook()
    partition_name = nc.partition_id_tensor.name if nc.partition_id_tensor else None
    in_names, out_names, out_avals, zero_outs = [], [], [], []
    for alloc in nc.m.functions[0].allocations:
        if not isinstance(alloc, mybir.MemoryLocationSet):
            continue
        name = alloc.memorylocations[0].name
        if alloc.kind == "ExternalInput":
            if name != partition_name:
                in_names.append(name)
        elif alloc.kind == "ExternalOutput":
            dt = mybir.dt.np(alloc.dtype)
            out_avals.append(jax.core.ShapedArray(tuple(alloc.tensor_shape), dt))
            out_names.append(name)
            zero_outs.append(np.zeros(tuple(alloc.tensor_shape), dt))
    n_params = len(in_names)
    bind_in_names = list(in_names) + list(out_names)
    if partition_name is not None:
        bind_in_names.append(partition_name)

    def _body(*args):
        operands = list(args)
        if partition_name is not None:
            operands.append(partition_id_tensor())
        outs = _bass_exec_p.bind(
            *operands,
            out_avals=tuple(out_avals),
            in_names=tuple(bind_in_names),
            out_names=tuple(out_names),
            lowering_input_output_aliases=(),
            sim_require_finite=True,
            sim_require_nnan=True,
            nc=nc,
        )
        return tuple(outs)

    devices = jax.devices()[:n_cores]
    mesh = Mesh(np.asarray(devices), ("core",))
    n_outs = len(out_names)
    in_specs = (PartitionSpec("core"),) * (n_params + n_outs)
    out_specs = (PartitionSpec("core"),) * n_outs
    fn = jax.jit(
        shard_map(
            _body, mesh=mesh, in_specs=in_specs, out_specs=out_specs, check_rep=False
        ),
        keep_unused=True,
    )
    per_core = [[np.asarray(m[name]) for name in in_names] for m in in_maps]
    concat_in = [
        np.concatenate([per_core[c][i] for c in range(n_cores)], axis=0)
        for i in range(n_params)
    ]
    concat_zeros = [
        np.zeros((n_cores * z.shape[0], *z.shape[1:]), z.dtype) for z in zero_outs
    ]
    sh = NamedSharding(mesh, PartitionSpec("core"))
    params_dev = [jax.device_put(a, sh) for a in concat_in]
    z = [jax.device_put(a, sh) for a in concat_zeros]
    outs = None
    for _ in range(3):
        outs = fn(*params_dev, *z)
    jax.block_until_ready(outs)

    def run(n):
        keep = None
        t0 = time.perf_counter()
        for _ in range(n):
            keep = fn(*params_dev, *z)
        jax.block_until_ready(keep)
        return time.perf_counter() - t0

    n1, n2 = max(5, n_iters // 5), n_iters
    t1 = run(n1)
    t2 = run(n2)
    per_iter = (t2 - t1) / (n2 - n1)
    print(
        f"benchmark: T({n1})={t1 * 1e3:.2f}ms T({n2})={t2 * 1e3:.2f}ms "
        f"slope={per_iter * 1e6:.1f}us/iter"
    )
    return per_iter
